# revision 1
# baseline (speedup 1.0000x reference)
"""Trainium2 Bass kernel for DiffCompressModule.

Reference computation (B=4, S=512, D_IN=D_OUT=4096):
    out = h @ W.T + b + coeff[b] * (h @ (2*mask[b] - 1))

Fused form used here (one matmul instead of two):
    out[b] = h[b] @ M_b + bias,   M_b = W.T + coeff[b] * (2*mask[b] - 1)

M_b is built in bf16 on ACT+DVE while the 256MB int32 mask streams from
HBM; the matmul runs in bf16 with fp32 PSUM accumulation. The kernel is
HBM-bound (~68MB per core).

Sharding over 8 cores: 4 out-feature groups x 2 batch groups.
Each core: h [2,512,4096], W [1024,4096], bias [1024], coeff [2],
mask [2,4096,1024] -> out [2,512,1024].
"""

import numpy as np

import concourse.bass as bass
import concourse.mybir as mybir
from concourse import tile, masks
from concourse.bass_utils import run_bass_kernel_spmd

B, S, D = 4, 512, 4096
O_FULL = 4096
N_CORES = 8
OG, BG = 4, 2  # out-feature groups x batch groups
O_SH = O_FULL // OG  # 1024 out features per core
B_SH = B // BG  # 2 batches per core
HALF = 512  # o processed in halves (PSUM/SBUF budget)
KC = D // 128  # 32 contraction chunks
SC = S // 128  # 4 s chunks
dt = mybir.dt

_CACHE = {}


def _split_sync_waits(nc, max_waits=1):
    # CoreV3 walrus rejects instructions with more than one semaphore wait
    # ("Too many sync wait commands"). Splitting the waits across preceding
    # same-engine NOPs is equivalent (the sequencer blocks on each in turn).
    ctr = 0
    for fn in nc.m.functions:
        for bb in fn.blocks:
            insts = bb.instructions
            if not any(
                i.sync_info is not None and len(i.sync_info.on_wait) > max_waits
                for i in insts
            ):
                continue
            new_list = []
            for ins in insts:
                si = ins.sync_info
                if si is not None and len(si.on_wait) > max_waits:
                    waits = list(si.on_wait)
                    head, tail = waits[:-max_waits], waits[-max_waits:]
                    for k in range(0, len(head), max_waits):
                        nop = mybir.InstNoOp(
                            name=f"waitsplit-{ctr}",
                            engine=ins.engine,
                            ins=[],
                            outs=[],
                            sync_info=mybir.SyncInfo(
                                on_wait=head[k : k + max_waits], on_update=[]
                            ),
                        )
                        ctr += 1
                        new_list.append(nop)
                    ins.sync_info = mybir.SyncInfo(
                        on_wait=tail, on_update=list(si.on_update)
                    )
                new_list.append(ins)
            bb.instructions = new_list


def _build_nc(loop_n=None):
    nc = bass.Bass("TRN2", target_bir_lowering=False, debug=False)
    h = nc.dram_tensor("h", [B_SH, D, S], dt.float32, kind="ExternalInput").ap()
    W = nc.dram_tensor("W", [D, O_SH], dt.float32, kind="ExternalInput").ap()
    bias = nc.dram_tensor("bias", [O_SH], dt.float32, kind="ExternalInput").ap()
    coeff = nc.dram_tensor("coeff", [B_SH], dt.float32, kind="ExternalInput").ap()
    mask = nc.dram_tensor("mask", [B_SH, D, O_SH], dt.int32, kind="ExternalInput").ap()
    out = nc.dram_tensor("out", [B_SH, S, O_SH], dt.float32, kind="ExternalOutput").ap()

    with tile.TileContext(nc) as tc:
        with (
            tc.tile_pool(name="const", bufs=1) as const_pool,
            tc.tile_pool(name="wt", bufs=KC // 2 + 1) as wt_pool,
            tc.tile_pool(name="ht", bufs=B_SH * KC // 4) as ht_pool,
            tc.tile_pool(name="mk", bufs=5) as mk_pool,
            tc.tile_pool(name="tt", bufs=2) as t_pool,
            tc.tile_pool(name="m", bufs=KC + 2) as m_pool,
            tc.tile_pool(name="ost", bufs=4) as out_pool,
            tc.tile_pool(name="acc", bufs=8, space="PSUM") as acc_pool,
        ):
            bias_bc = const_pool.tile([128, O_SH], dt.float32)
            nc.sync.dma_start(
                bias_bc[:], bass.AP(bias.tensor, 0, [[0, 128], [1, O_SH]])
            )
            coeff_bc = const_pool.tile([128, B_SH], dt.float32)
            nc.sync.dma_start(
                coeff_bc[:], bass.AP(coeff.tensor, 0, [[0, 128], [1, B_SH]])
            )
            c2 = const_pool.tile([128, B_SH], dt.float32)
            cneg = const_pool.tile([128, B_SH], dt.float32)
            nc.vector.tensor_scalar_mul(c2[:], coeff_bc[:], 2.0)
            nc.vector.tensor_scalar_mul(cneg[:], coeff_bc[:], -1.0)

            ht = {}

            import contextlib

            loop_ctx = (
                tc.For_i(
                    0,
                    loop_n,
                    1,
                    hint_engines=(
                        mybir.EngineType.PE,
                        mybir.EngineType.Activation,
                        mybir.EngineType.DVE,
                        mybir.EngineType.SP,
                        mybir.EngineType.Pool,
                    ),
                )
                if loop_n
                else contextlib.nullcontext()
            )

            def build_ht_kg(b, kg):
                # h arrives pre-transposed [b, i, s] from the host: one
                # casting SWDGE DMA fills 4 kc chunks ([128 i, 4*512 s] bf16)
                if kg % 2 == 1:
                    return
                for q in range(4):  # 4 quads cover kc in [kg*8, kg*8+16)
                    kc0 = kg * 8 + q * 4
                    ht4 = ht_pool.tile([128, 4 * S], dt.bfloat16, name="ht4")
                    nc.gpsimd.dma_start(
                        ht4[:],
                        bass.AP(
                            h.tensor,
                            (b * D + kc0 * 128) * S,
                            [[S, 128], [128 * S, 4], [1, S]],
                        ),
                    )
                    for j in range(4):
                        ht[(b, kc0 + j)] = ht4[:, j * S : (j + 1) * S]

            def build_wt_kg(half, kg, wt):
                # W arrives pre-transposed [i, o] from the host: one casting
                # SWDGE DMA fills 4 kc chunks ([128 i, 4*512 o] bf16)
                o0 = half * HALF
                for q in range(2):  # 2 quads cover kc in [kg*8, kg*8+8)
                    kc0 = kg * 8 + q * 4
                    wt4 = wt_pool.tile([128, 4 * HALF], dt.bfloat16, name="wt4")
                    nc.gpsimd.dma_start(
                        wt4[:],
                        bass.AP(
                            W.tensor,
                            kc0 * 128 * O_SH + o0,
                            [[O_SH, 128], [128 * O_SH, 4], [1, HALF]],
                        ),
                    )
                    for j in range(4):
                        wt.append(wt4[:, j * HALF : (j + 1) * HALF])

            def round_kg(half, b, kg, wt, accs):
                o0 = half * HALF
                for k2 in range(2):  # quads of kc chunks, cast int32->bf16 in DMA
                    kc0 = kg * 8 + k2 * 4
                    mk = mk_pool.tile([128, 4 * HALF], dt.bfloat16, name="mk")
                    nc.gpsimd.dma_start(
                        mk[:],
                        bass.AP(
                            mask.tensor,
                            (b * D + kc0 * 128) * O_SH + o0,
                            [[O_SH, 128], [128 * O_SH, 4], [1, HALF]],
                        ),
                    )
                    t_sb = t_pool.tile([128, 4 * HALF], dt.bfloat16, name="tsb")
                    nc.scalar.activation(
                        t_sb[:],
                        mk[:],
                        mybir.ActivationFunctionType.Identity,
                        bias=cneg[:, b : b + 1],
                        scale=c2[:, b : b + 1],
                    )
                    for j in range(4):
                        kc = kc0 + j
                        m = m_pool.tile([128, HALF], dt.bfloat16, name="m")
                        nc.vector.tensor_tensor(
                            m[:],
                            t_sb[:, j * HALF : (j + 1) * HALF],
                            wt[kc][:],
                            mybir.AluOpType.add,
                        )
                        for sc in range(SC):
                            htap = ht[(b, kc)]
                            nc.tensor.matmul(
                                accs[sc][:],
                                htap[:, sc * 128 : (sc + 1) * 128],
                                m[:],
                                start=(kc == 0),
                                stop=(kc == KC - 1),
                            )

            def epilogue(half, b, accs):
                o0 = half * HALF
                for sc in range(SC):
                    o_sb = out_pool.tile([128, HALF], dt.float32, name="osb")
                    nc.vector.tensor_tensor(
                        o_sb[:],
                        accs[sc][:],
                        bias_bc[:, o0 : o0 + HALF],
                        mybir.AluOpType.add,
                    )
                    nc.sync.dma_start(
                        out[b, sc * 128 : (sc + 1) * 128, o0 : o0 + HALF], o_sb[:]
                    )

            def new_accs():
                return [
                    acc_pool.tile([128, HALF], dt.float32, tag="acc", name="acc")
                    for _ in range(SC)
                ]

            with loop_ctx:
                wt0, wt1 = [], []
                accs = new_accs()
                for kg in range(4):
                    build_ht_kg(0, kg)
                    build_wt_kg(0, kg, wt0)
                    round_kg(0, 0, kg, wt0, accs)
                    build_wt_kg(1, kg, wt1)
                epilogue(0, 0, accs)
                accs = new_accs()
                for kg in range(4):
                    round_kg(1, 0, kg, wt1, accs)
                    build_ht_kg(1, kg)
                epilogue(1, 0, accs)
                accs = new_accs()
                for kg in range(4):
                    round_kg(0, 1, kg, wt0, accs)
                epilogue(0, 1, accs)
                accs = new_accs()
                for kg in range(4):
                    round_kg(1, 1, kg, wt1, accs)
                epilogue(1, 1, accs)

    _split_sync_waits(nc)
    return nc


def _get_nc():
    if "nc" not in _CACHE:
        _CACHE["nc"] = _build_nc()
    return _CACHE["nc"]


def kernel(hidden_states, W, b, coeff, mask, _trace=False, _trace_kwargs=None):
    nc = _get_nc()
    in_maps = []
    for core in range(N_CORES):
        g, bj = core // BG, core % BG
        in_maps.append(
            {
                "h": np.ascontiguousarray(
                    np.asarray(hidden_states)[
                        bj * B_SH : (bj + 1) * B_SH
                    ].transpose(0, 2, 1),
                    dtype=np.float32,
                ),
                "W": np.ascontiguousarray(
                    np.asarray(W)[g * O_SH : (g + 1) * O_SH].T, dtype=np.float32
                ),
                "bias": np.ascontiguousarray(
                    b[g * O_SH : (g + 1) * O_SH], dtype=np.float32
                ),
                "coeff": np.ascontiguousarray(
                    coeff[bj * B_SH : (bj + 1) * B_SH], dtype=np.float32
                ),
                "mask": np.ascontiguousarray(
                    mask[bj * B_SH : (bj + 1) * B_SH, :, g * O_SH : (g + 1) * O_SH],
                    dtype=np.int32,
                ),
            }
        )
    kwargs = {}
    if _trace:
        kwargs = {"trace": True, "trace_kwargs": _trace_kwargs or {}}
    # The first touch of the device after an abnormal process exit can fail
    # with NRT_EXEC_UNIT_UNRECOVERABLE; the failed attempt clears the wedged
    # state, so retry.
    last_err = None
    for attempt in range(3):
        try:
            res = run_bass_kernel_spmd(
                nc, in_maps, core_ids=list(range(N_CORES)), **kwargs
            )
            break
        except Exception as e:  # jax.errors.JaxRuntimeError etc.
            last_err = e
            try:
                import jax

                jax.clear_caches()
            except Exception:
                pass
            import time as _time

            _time.sleep(2.0)
    else:
        raise last_err
    _CACHE["last_results"] = res

    out = np.empty((B, S, O_FULL), dtype=np.float32)
    for core in range(N_CORES):
        g, bj = core // BG, core % BG
        out[bj * B_SH : (bj + 1) * B_SH, :, g * O_SH : (g + 1) * O_SH] = res.results[
            core
        ]["out"]
    return out



# revision 28
# speedup vs baseline: 1.1679x; 1.1679x over previous
"""Trainium2 Bass kernel for DiffCompressModule.

Reference computation (B=4, S=512, D_IN=D_OUT=4096):
    out = h @ W.T + b + coeff[b] * (h @ (2*mask[b] - 1))

Fused form used here (one matmul instead of two):
    out[b] = h[b] @ M_b + bias,   M_b = W.T + coeff[b] * (2*mask[b] - 1)

Host prep is layout/encoding only: h and W are pre-transposed and cast to
bf16 (the matmul runs in bf16 anyway), and the {0,1} int32 mask is
re-encoded as int8 (values preserved exactly).  On-chip, ACT decodes the
mask to coeff-scaled deltas (2c*mask - c, int8 -> bf16) while DVE adds
W.T; the fused matmul accumulates in fp32 PSUM.

Sharding over 8 cores: 4 out-feature groups x 2 batch groups.
Each core: h [2,4096,512] bf16, W [4096,1024] bf16, bias [1024] f32,
coeff [2] f32, mask [2,4096,1024] int8 -> out [2,512,1024] bf16
(upcast to fp32 on the host).

A burst of tiny warm-up matmuls on dummy tiles runs while the first mask
tiles stream in, so the PE p-state ramp is paid during the DMA head
instead of on the real matmuls.
"""

import numpy as np

import concourse.bass as bass
import concourse.mybir as mybir
from concourse import tile
from concourse.bass_utils import run_bass_kernel_spmd

B, S, D = 4, 512, 4096
O_FULL = 4096
N_CORES = 8
OG, BG = 4, 2  # out-feature groups x batch groups
O_SH = O_FULL // OG  # 1024 out features per core
B_SH = B // BG  # 2 batches per core
HALF = 512  # o processed in halves (one PSUM bank per acc tile)
KC = D // 128  # 32 contraction chunks
KG = KC // 4  # 8 groups of 4 contraction chunks
SC = S // 128  # 4 s chunks
WARMUP = 64  # PE warm-up matmuls (N=128) overlapping the DMA head
dt = mybir.dt

_CACHE = {}


def _split_sync_waits(nc, max_waits=1):
    # CoreV3 walrus rejects instructions with more than one semaphore wait
    # ("Too many sync wait commands"). Splitting the waits across preceding
    # same-engine NOPs is equivalent (the sequencer blocks on each in turn).
    ctr = 0
    for fn in nc.m.functions:
        for bb in fn.blocks:
            insts = bb.instructions
            if not any(
                i.sync_info is not None and len(i.sync_info.on_wait) > max_waits
                for i in insts
            ):
                continue
            new_list = []
            for ins in insts:
                si = ins.sync_info
                if si is not None and len(si.on_wait) > max_waits:
                    waits = list(si.on_wait)
                    head, tail = waits[:-max_waits], waits[-max_waits:]
                    for k in range(0, len(head), max_waits):
                        nop = mybir.InstNoOp(
                            name=f"waitsplit-{ctr}",
                            engine=ins.engine,
                            ins=[],
                            outs=[],
                            sync_info=mybir.SyncInfo(
                                on_wait=head[k : k + max_waits], on_update=[]
                            ),
                        )
                        ctr += 1
                        new_list.append(nop)
                    ins.sync_info = mybir.SyncInfo(
                        on_wait=tail, on_update=list(si.on_update)
                    )
                new_list.append(ins)
            bb.instructions = new_list


def _build_nc(warmup=WARMUP):
    nc = bass.Bass("TRN2", target_bir_lowering=False, debug=False,
                   dynamic_dma_scratch_size=2048)
    h = nc.dram_tensor("h", [B_SH, D, S], dt.bfloat16, kind="ExternalInput").ap()
    W = nc.dram_tensor("W", [D, O_SH], dt.bfloat16, kind="ExternalInput").ap()
    bias = nc.dram_tensor("bias", [O_SH], dt.bfloat16, kind="ExternalInput").ap()
    cpre = nc.dram_tensor("cpre", [2 * B_SH], dt.float32, kind="ExternalInput").ap()
    mask = nc.dram_tensor("mask", [B_SH, D, O_SH], dt.int8, kind="ExternalInput").ap()
    out = nc.dram_tensor("out", [B_SH, S, O_SH], dt.bfloat16, kind="ExternalOutput").ap()

    with tile.TileContext(nc) as tc:
        with (
            tc.tile_pool(name="const", bufs=1) as const_pool,
            tc.tile_pool(name="wt", bufs=KG) as wt_pool,
            tc.tile_pool(name="ht", bufs=2 * KG) as ht_pool,
            tc.tile_pool(name="mk", bufs=8) as mk_pool,
            tc.tile_pool(name="m", bufs=4) as m_pool,
            tc.tile_pool(name="ost", bufs=8) as out_pool,
            tc.tile_pool(name="acc", bufs=8, space="PSUM") as acc_pool,
        ):
            # PE warm-up: tiny matmuls on a dummy tile, no input dependency;
            # they pay the p-state ramp during the DMA/pipeline-fill head and
            # hand off to the real matmul stream with no gap.
            if warmup:
                wa = const_pool.tile([128, 128], dt.bfloat16)
                nc.vector.memset(wa[:], 0.0)
                wacc = acc_pool.tile([128, HALF], dt.float32, tag="acc", name="wacc")
                for i in range(warmup):
                    nc.tensor.matmul(
                        wacc[:, :128], wa[:], wa[:],
                        start=(i == 0), stop=(i == warmup - 1),
                    )

            cpre_bc = const_pool.tile([128, 2 * B_SH], dt.float32)
            nc.sync.dma_start(
                cpre_bc[:], bass.AP(cpre.tensor, 0, [[0, 128], [1, 2 * B_SH]])
            )
            c2 = cpre_bc[:, 0:B_SH]
            cneg = cpre_bc[:, B_SH : 2 * B_SH]
            bias_bc = const_pool.tile([128, O_SH], dt.bfloat16)

            def load_bias():
                nc.sync.dma_start(
                    bias_bc[:], bass.AP(bias.tensor, 0, [[0, 128], [1, O_SH]])
                )

            wt = {}  # kg -> [128, 4*O_SH] bf16 (4 kc chunks, full o width)
            ht = {}  # (b, kg) -> [128, 4*S] bf16
            pending_mk = {}  # (b, kg) -> mask tile with DMA issued

            def load_wt(kg, j0=0, nj=4):
                if kg not in wt:
                    wt[kg] = wt_pool.tile([128, 4 * O_SH], dt.bfloat16, name="w4")
                nc.sync.dma_start(
                    wt[kg][:, j0 * O_SH : (j0 + nj) * O_SH],
                    bass.AP(
                        W.tensor,
                        (kg * 4 + j0) * 128 * O_SH,
                        [[O_SH, 128], [128 * O_SH, nj], [1, O_SH]],
                    ),
                )

            def load_ht(b, kg, j0=0, nj=4):
                if (b, kg) not in ht:
                    ht[(b, kg)] = ht_pool.tile([128, 4 * S], dt.bfloat16, name="h4")
                nc.sync.dma_start(
                    ht[(b, kg)][:, j0 * S : (j0 + nj) * S],
                    bass.AP(
                        h.tensor,
                        (b * D + (kg * 4 + j0) * 128) * S,
                        [[S, 128], [128 * S, nj], [1, S]],
                    ),
                )

            def mask_dma(b, kg, j0=0, nj=4, mk=None, eng=None):
                if mk is None:
                    mk = mk_pool.tile([128, 4 * O_SH], dt.int8, name="mk")
                (eng or nc.sync).dma_start(
                    mk[:, j0 * O_SH : (j0 + nj) * O_SH],
                    bass.AP(
                        mask.tensor,
                        (b * D + (kg * 4 + j0) * 128) * O_SH,
                        [[O_SH, 128], [128 * O_SH, nj], [1, O_SH]],
                    ),
                )
                return mk

            def queue_mask(b, kg):
                if (b, kg) not in pending_mk:
                    pending_mk[(b, kg)] = mask_dma(b, kg)

            def build_m(b, mk, m4, j0=0, nj=4):
                # delta = 2c*mask - c (int8 -> bf16 on ACT), then m += W.T in
                # place on DVE
                nc.scalar.activation(
                    m4[:, j0 * O_SH : (j0 + nj) * O_SH],
                    mk[:, j0 * O_SH : (j0 + nj) * O_SH],
                    mybir.ActivationFunctionType.Identity,
                    bias=cpre_bc[:, B_SH + b : B_SH + b + 1],
                    scale=cpre_bc[:, b : b + 1],
                )

            def add_w(kg, m4, j0=0, nj=4):
                sl = slice(j0 * O_SH, (j0 + nj) * O_SH)
                nc.vector.tensor_tensor(
                    m4[:, sl], m4[:, sl], wt[kg][:, sl], mybir.AluOpType.add
                )

            def mms(b, kg, j, accs, m4, hf_sc=None):
                kc = kg * 4 + j
                h4 = ht[(b, kg)]
                order = hf_sc if hf_sc is not None else [
                    (hf, sc) for hf in range(2) for sc in range(SC)
                ]
                for hf, sc in order:
                    nc.tensor.matmul(
                        accs[hf * SC + sc][:],
                        h4[:, j * S + sc * 128 : j * S + (sc + 1) * 128],
                        m4[:, j * O_SH + hf * HALF : j * O_SH + hf * HALF + HALF],
                        start=(kc == 0),
                        stop=(kc == KC - 1),
                    )

            def make_m4(b, kg, prefetch=(), split=False):
                mk = pending_mk.pop((b, kg), None)
                if mk is None:
                    mk = mask_dma(b, kg)
                for fn in prefetch:
                    fn()
                m4 = m_pool.tile([128, 4 * O_SH], dt.bfloat16, name="m4")
                if split:
                    build_m(b, mk, m4, 0, 2)
                    add_w(kg, m4, 0, 2)
                    build_m(b, mk, m4, 2, 2)
                    add_w(kg, m4, 2, 2)
                else:
                    build_m(b, mk, m4)
                    add_w(kg, m4)
                return m4

            def round_kg(b, kg, accs, prefetch=(), m4=None, bank_major=False,
                         split=False):
                if m4 is None:
                    m4 = make_m4(b, kg, prefetch, split=split)
                else:
                    for fn in prefetch:
                        fn()
                if bank_major:
                    # first group after a pass boundary: touch the PSUM banks
                    # in release order so each matmul starts right as its
                    # bank's previous-pass epilogue frees it
                    for i in range(2 * SC):
                        hf, sc = i // SC, i % SC
                        for j in range(4):
                            mms(b, kg, j, accs, m4, hf_sc=[(hf, sc)])
                else:
                    for j in range(4):
                        mms(b, kg, j, accs, m4)

            def fine_fill(b, accs):
                # pipeline fill: per piece emit mask -> W -> h on the single
                # SP queue (exact consumption order into the DMA engines),
                # then affine/add/matmuls.  Granularity 1,1,1,1 / 2,2 / 2,2
                # over kg0-2; steady full-kg rounds afterwards.
                pieces = [(0, 0, 1), (0, 1, 1), (0, 2, 1), (0, 3, 1),
                          (1, 0, 2), (1, 2, 2), (2, 0, 2), (2, 2, 2)]
                mks = {}
                m4s = {}
                for kg in (0, 1, 2):
                    mks[kg] = mk_pool.tile([128, 4 * O_SH], dt.int8, name="mk")
                    m4s[kg] = m_pool.tile([128, 4 * O_SH], dt.bfloat16, name="m4")
                # mask stream runs 4 pieces ahead of the W/h stream so
                # the ACT affine chain never waits on a queued mask transfer
                for kg, j0, nj in pieces[:4]:
                    mask_dma(b, kg, j0, nj, mk=mks[kg], eng=nc.sync)
                for pi, (kg, j0, nj) in enumerate(pieces):
                    if pi + 4 < len(pieces):
                        nkg, nj0, nnj = pieces[pi + 4]
                        mask_dma(b, nkg, nj0, nnj, mk=mks[nkg], eng=nc.sync)
                    load_wt(kg, j0, nj)
                    load_ht(b, kg, j0, nj)
                    if pi == 4:
                        queue_mask(b, 3)
                    if pi == 6:
                        load_wt(3)
                        load_ht(b, 3)
                    if pi == 7:
                        queue_mask(b, 4)
                        load_wt(4)
                        load_ht(b, 4)
                        load_ht(b, 5, 0, 2)
                    build_m(b, mks[kg], m4s[kg], j0, nj)
                    add_w(kg, m4s[kg], j0, nj)
                    for j in range(j0, j0 + nj):
                        mms(b, kg, j, accs, m4s[kg])

            EPI_ENGINES = (nc.vector,) * 8

            def epi_thunks(b, accs, engines=None):
                def mk_thunk(i):
                    hf, sc = i // SC, i % SC

                    def run():
                        eng = (engines or EPI_ENGINES)[i]
                        o0 = hf * HALF
                        o_sb = out_pool.tile([128, HALF], dt.bfloat16, name="osb")
                        eng.tensor_tensor(
                            o_sb[:],
                            accs[i][:],
                            bias_bc[:, o0 : o0 + HALF],
                            mybir.AluOpType.add,
                        )
                        nc.sync.dma_start(
                            out[b, sc * 128 : (sc + 1) * 128, o0 : o0 + HALF],
                            o_sb[:],
                        )
                    return run

                return [mk_thunk(i) for i in range(2 * SC)]

            def round_kg_last(b, kg, accs, prefetch=(), pre_build=()):
                # bank-major final group: each acc bank finishes early and its
                # epilogue runs while the remaining banks' matmuls continue.
                # pre_build emits the next pass's first m4 chains before the
                # epilogues so they never gate the m pipeline.
                m4 = make_m4(b, kg, prefetch)
                built = [fn() for fn in pre_build]
                epi = epi_thunks(b, accs)
                for i in range(2 * SC):
                    hf, sc = i // SC, i % SC
                    for j in range(4):
                        mms(b, kg, j, accs, m4, hf_sc=[(hf, sc)])
                    epi[i]()
                return built

            def new_accs():
                return [
                    acc_pool.tile([128, HALF], dt.float32, tag="acc", name="acc")
                    for _ in range(2 * SC)
                ]

            # pass 1: batch 0
            accs = new_accs()
            fine_fill(0, accs)
            for kg in range(3, KG - 1):
                pf = []
                if kg + 2 < KG:
                    pf.append(lambda kg=kg: queue_mask(0, kg + 2))
                if kg + 2 < KG:
                    pf += [lambda kg=kg: load_wt(kg + 2, 0, 2),
                           lambda kg=kg: load_ht(0, kg + 2, 0, 2),
                           lambda kg=kg: load_wt(kg + 2, 2, 2),
                           lambda kg=kg: load_ht(0, kg + 2, 2, 2)]
                if kg == 3:
                    pf.append(load_bias)
                round_kg(0, kg, accs, prefetch=tuple(pf), split=True)
            pre = round_kg_last(
                0, KG - 1, accs,
                prefetch=(lambda: queue_mask(1, 0), lambda: queue_mask(1, 1),
                          lambda: load_ht(1, 0), lambda: load_ht(1, 1)),
                pre_build=(lambda: make_m4(1, 0), lambda: make_m4(1, 1)),
            )
            # pass 2: batch 1
            accs = new_accs()
            for kg in range(KG - 1):
                pf = []
                if kg + 2 < KG:
                    pf += [lambda kg=kg: queue_mask(1, kg + 2),
                           lambda kg=kg: load_ht(1, kg + 2)]
                round_kg(1, kg, accs, prefetch=tuple(pf),
                         m4=pre[kg] if kg < len(pre) else None,
                         bank_major=(kg == 0))
            round_kg_last(1, KG - 1, accs)

    _split_sync_waits(nc)
    return nc


def _get_nc():
    if "nc" not in _CACHE:
        _CACHE["nc"] = _build_nc()
    return _CACHE["nc"]


def kernel(hidden_states, W, b, coeff, mask, _trace=False, _trace_kwargs=None):
    import ml_dtypes

    bf16 = ml_dtypes.bfloat16
    nc = _get_nc()
    hidden_states = np.asarray(hidden_states)
    W = np.asarray(W)
    mask_i8 = np.asarray(mask).astype(np.int8)
    in_maps = []
    for core in range(N_CORES):
        g, bj = core // BG, core % BG
        in_maps.append(
            {
                "h": np.ascontiguousarray(
                    hidden_states[bj * B_SH : (bj + 1) * B_SH].transpose(0, 2, 1)
                ).astype(bf16),
                "W": np.ascontiguousarray(
                    W[g * O_SH : (g + 1) * O_SH].T
                ).astype(bf16),
                "bias": np.ascontiguousarray(
                    b[g * O_SH : (g + 1) * O_SH], dtype=np.float32
                ).astype(bf16),
                "cpre": np.concatenate([
                    2.0 * np.asarray(coeff[bj * B_SH : (bj + 1) * B_SH], dtype=np.float32),
                    -np.asarray(coeff[bj * B_SH : (bj + 1) * B_SH], dtype=np.float32),
                ]),
                "mask": np.ascontiguousarray(
                    mask_i8[bj * B_SH : (bj + 1) * B_SH, :, g * O_SH : (g + 1) * O_SH]
                ),
            }
        )
    kwargs = {}
    if _trace:
        kwargs = {"trace": True, "trace_kwargs": _trace_kwargs or {}}
    # The first touch of the device after an abnormal process exit can fail
    # with NRT_EXEC_UNIT_UNRECOVERABLE; the failed attempt clears the wedged
    # state, so retry.
    last_err = None
    for attempt in range(3):
        try:
            res = run_bass_kernel_spmd(
                nc, in_maps, core_ids=list(range(N_CORES)), **kwargs
            )
            break
        except Exception as e:  # jax.errors.JaxRuntimeError etc.
            last_err = e
            try:
                import jax

                jax.clear_caches()
            except Exception:
                pass
            import time as _time

            _time.sleep(2.0)
    else:
        raise last_err
    _CACHE["last_results"] = res

    out = np.empty((B, S, O_FULL), dtype=np.float32)
    for core in range(N_CORES):
        g, bj = core // BG, core % BG
        out[bj * B_SH : (bj + 1) * B_SH, :, g * O_SH : (g + 1) * O_SH] = np.asarray(
            res.results[core]["out"]
        ).astype(np.float32)
    return out
